# revision 23
# baseline (speedup 1.0000x reference)
"""Hierarchical-softmax loss kernel for Trainium2 (8 NeuronCores).

Strategy (v3: host-staged fp8 TensorE all-pairs)
------------------------------------------------
Data-parallel over n_ex: examples are globally sorted by path length and
dealt round-robin so every core sees 8 partition-tiles of 128 examples
with a near-common per-tile max length lm.

Per tile, the dot products are computed as a DENSE fp8 matmul on the
Tensor engine: stationary = the tile's 128 x-vectors (fp8, d on
partitions, parity-interleaved), moving = the tile's 128*lm signed path
rows (-code * 512 * W[node]).  The host pre-gathers those rows and
writes them to DRAM ALREADY in the transposed, parity-interleaved SBUF
byte layout the PE wants, so the device only runs plain full-rate
streaming DMAs (no indirect gather, no Q7 descriptor generation) and 4
DoubleRow fp8 matmuls per PSUM bank accumulate the full D=1024
contraction at 2 k-rows/cycle.

psum[e, col] holds x[e] . w_col for ALL (example, path-column) pairs of
the tile; only the block-diagonal (col belongs to e) is wanted. A bf16
bias table (0 on-path, -16384 off-path) added on the Vector engine sends
off-path entries to softplus(-32) ~ 1e-14, so the Scalar engine's
exp(v/512) -> ln(1+u) pass with free-axis accumulation per tile yields
the masked loss partials directly. Host sums the 128x8 partials/core.

All flops of the loss (dots, softplus, reduction) stay on device; the
host only reorders bytes. Per-core HBM traffic ~19MB, all streaming.
"""

import os
import sys

import numpy as np

for _p in ("/opt/trn_rl_repo", "/root/.axon_site/_ro/trn_rl_repo"):
    if os.path.isdir(_p) and _p not in sys.path:
        sys.path.append(_p)

V = 50257
N_DEC = V - 1
D = 1024
N_EX = 8192
MAX_LEN = 24
N_CORES = 8
P = 128
N_TILES = N_EX // (N_CORES * P)  # 8 example-tiles of 128 per core
W_SCALE = 512.0                  # fp8 pre-scale
OFF_BIAS = -16384.0              # off-path bias; /512 = -32 => softplus ~ 1e-14
GRP = 512                        # psum bank columns (fp32)
_prog_cache: dict = {}


def _patch_tail_drain(tile, mybir, bass_rust):
    """The pinned walrus encodes only a limited number of sync-waits per CTRL
    instruction, but Tile's kernel-tail Drain carries one wait per active
    processor lane. Spread the extra waits over single-wait NOPs."""
    if getattr(tile.TileContext._drain_and_barrier, "_split_waits", False):
        return

    def _drain_and_barrier(self, tick_clock, wait_clock):
        nc = self.nc
        drain_inst = nc.sync.drain()
        wait_clock.add_sem_waits(
            drain_inst.ins, bass_rust.ScopedClock({None: tick_clock.global_clock})
        )
        si = drain_inst.ins.sync_info
        waits = list(si.on_wait or [])
        if len(waits) > 1:
            si.on_wait = waits[:1]
            for w in waits[1:]:
                nop = nc.sync.nop(nofuse=True)
                nop.ins.sync_info = mybir.SyncInfo(on_wait=[w], on_update=[])
        nc.all_engine_barrier()
        popped = nc._tile_sem_poison_stack.pop()
        assert popped is self._sem_poison
        nc.clear_and_free_semaphores(list(self.sems.allocated().values()))
        nc.all_engine_barrier()

    _drain_and_barrier._split_waits = True
    tile.TileContext._drain_and_barrier = _drain_and_barrier


def _split_multiwait_instructions(nc, mybir, maxw=1):
    """Hoist extra sem-waits from any instruction onto single-wait NOPs placed
    immediately before it on the same engine (same aggregate wait semantics)."""
    f = nc.m.functions[0]
    tail = nc.cur_bb.bb
    blocks = list(f.blocks)
    if not any(b.name == tail.name for b in blocks):
        blocks.append(tail)
    for blk in blocks:
        snapshot = list(blk.instructions)
        heavy = [
            i for i in snapshot
            if i.sync_info and i.sync_info.on_wait and len(i.sync_info.on_wait) > maxw
        ]
        if not heavy:
            continue
        pre_len = len(tail.instructions)
        n_created = 0
        new_list = []
        for inst in snapshot:
            si = inst.sync_info
            if si and si.on_wait and len(si.on_wait) > maxw:
                waits = list(si.on_wait)
                extra, keep = waits[:-maxw], waits[-maxw:]
                si.on_wait = keep
                for w in extra:
                    nop = nc.engines[inst.engine].nop(nofuse=True)
                    nop.ins.sync_info = mybir.SyncInfo(on_wait=[w], on_update=[])
                    new_list.append(nop.ins)
                    n_created += 1
            new_list.append(inst)
        # builder appended the fresh NOPs to the tail block; strip them there
        t = list(tail.instructions)
        assert len(t) == pre_len + n_created
        if blk.name == tail.name:
            blk.instructions = new_list
        else:
            tail.instructions = t[:pre_len]
            blk.instructions = new_list


def _build_program(lmax: tuple):
    from concourse import bass, mybir
    import concourse.tile as tile
    import bass_rust

    _patch_tail_drain(tile, mybir, bass_rust)

    F = [P * lm for lm in lmax]        # pair-columns per tile
    Ftot = sum(F)
    NPARTS = sum((f + 1023) // 1024 for f in F)

    nc = bass.Bass("TRN2", target_bir_lowering=False)
    f32 = mybir.dt.float32
    bf16 = mybir.dt.bfloat16
    fp8 = mybir.dt.float8e4
    fp8b = mybir.dt.float8e5   # bias: {0, -16384}, both exact in e5m2

    # Wt: host-transposed signed rows; per partition p, tile k, the byte at
    # [c4*2*Fk + 2i + j] is W-element d = 2*(c4*128+p)+j of tile-k column i
    Wt = nc.declare_dram_parameter("Wt", [P, 8 * Ftot], fp8, isOutput=False)
    xst = nc.declare_dram_parameter("xst", [P, N_TILES * 1024], fp8, isOutput=False)
    bias = nc.declare_dram_parameter("bias", [P, Ftot], fp8b, isOutput=False)
    out = nc.declare_dram_parameter("out", [P, NPARTS], f32, isOutput=True)

    with tile.TileContext(nc) as tc:
        with (
            tc.tile_pool(name="meta", bufs=1) as meta,
            tc.tile_pool(name="gpool", bufs=1) as gpool,
            tc.tile_pool(name="bpool", bufs=1) as bpool,
            tc.tile_pool(name="vpool", bufs=2) as vpool,
            tc.tile_pool(name="spool", bufs=2) as spool,
            tc.psum_pool(name="pspool", bufs=1) as pspool,
            tc.tile_pool(name="outp", bufs=1) as outp,
        ):
            xst_t = meta.tile([P, N_TILES * 1024], fp8, tag="xst")
            nc.scalar.dma_start(out=xst_t[:], in_=xst[:, :])

            # parts: one accumulator column per ~1024-column ACT piece
            n_piece = [(F[k] + 1023) // 1024 for k in range(N_TILES)]
            parts = outp.tile([P, sum(n_piece)], f32, tag="parts")

            # smallest tile first: compute starts right behind the DMA
            # stream; the tail after the last DMA is the biggest tile's
            # final piece only
            order = sorted(range(N_TILES), key=lambda k: lmax[k])
            f_off = [sum(F[:k]) for k in range(N_TILES)]
            p_off = [sum(n_piece[:k]) for k in range(N_TILES)]

            # everything fits in SBUF (~170KB/partition): keep all tiles
            # persistent and issue every DMA upfront so the Wt stream
            # never stalls on a buffer rotation
            gt, bt = {}, {}
            for ki, k in enumerate(order):
                Fk = F[k]
                g = gpool.tile([P, 8, Fk], fp8, tag=f"g{k}", name=f"g{k}")
                weng = nc.sync if ki % 2 == 0 else nc.scalar
                weng.dma_start(
                    out=g.rearrange("p a b -> p (a b)"),
                    in_=Wt[:, 8 * f_off[k] : 8 * (f_off[k] + Fk)],
                )
                b = bpool.tile([P, Fk], fp8b, tag=f"b{k}", name=f"b{k}")
                beng = nc.scalar if ki % 2 == 0 else nc.sync
                beng.dma_start(out=b[:], in_=bias[:, f_off[k] : f_off[k] + Fk])
                gt[k], bt[k] = g, b

            for k in order:
                Fk = F[k]
                gflat = gt[k].rearrange("p a b -> p (a b)")
                bias_t = bt[k]
                v = vpool.tile([P, Fk], bf16, tag="v")
                n_grp = (Fk + GRP - 1) // GRP
                # group-level pipeline: each 512-col group's psum completes
                # (4 accumulating chunk-matmuls), its bias-add fires, and
                # every 2 groups the ScalarE softplus piece runs
                for gi in range(n_grp):
                    j0 = gi * GRP
                    Fg = min(GRP, Fk - j0)
                    ps = pspool.tile(
                        [P, GRP], f32, tag=f"ps{gi % 6}", name=f"ps{gi % 6}"
                    )
                    for c in range(4):
                        rhs = gflat[
                            :, c * 2 * Fk + 2 * j0 : c * 2 * Fk + 2 * (j0 + Fg)
                        ].rearrange("p (f two) -> p two f", two=2)
                        lhsT = xst_t[
                            :, k * 1024 + c * 256 : k * 1024 + (c + 1) * 256
                        ].rearrange("p (two e) -> p two e", two=2)
                        nc.tensor.matmul(
                            out=ps[:, :Fg],
                            lhsT=lhsT,
                            rhs=rhs,
                            start=(c == 0),
                            stop=(c == 3),
                            perf_mode=mybir.MatmulPerfMode.DoubleRow,
                        )
                    nc.vector.tensor_tensor(
                        out=v[:, j0 : j0 + Fg],
                        in0=ps[:, :Fg],
                        in1=bias_t[:, j0 : j0 + Fg],
                        op=mybir.AluOpType.add,
                    )
                    if gi % 2 == 1 or gi == n_grp - 1:
                        # softplus piece over the last 1-2 groups:
                        # ln(1 + exp(v/512)); off-path v/512 = -32
                        h0 = (gi // 2) * 1024
                        Fh = j0 + Fg - h0
                        pc = p_off[k] + gi // 2
                        sp = spool.tile([P, 1024], bf16, tag="sp")
                        nc.scalar.activation(
                            out=sp[:, :Fh],
                            in_=v[:, h0 : h0 + Fh],
                            func=mybir.ActivationFunctionType.Exp,
                            scale=1.0 / W_SCALE,
                        )
                        sl = spool.tile([P, 1024], bf16, tag="sl")
                        nc.scalar.activation(
                            out=sl[:, :Fh],
                            in_=sp[:, :Fh],
                            func=mybir.ActivationFunctionType.Ln,
                            bias=1.0,
                            accum_out=parts[:, pc : pc + 1],
                        )
            nc.sync.dma_start(out=out[:, :], in_=parts[:])

    _split_multiwait_instructions(nc, mybir)
    return nc


def _prepare(x, W, t, paths, codes, lens):
    """Host-side prep: length-sorted round-robin shard; per-core pre-gathered,
    pre-signed, pre-transposed fp8 W path-rows; x stationaries; bias tables."""
    import ml_dtypes

    L = lens[t].astype(np.int64)                      # [N_EX]
    rank = np.argsort(-L, kind="stable")              # examples by length desc

    # slot s (0..1023) of core c takes example rank[s*8 + c]
    sel = rank.reshape(N_EX // N_CORES, N_CORES)      # [1024, 8]
    lmax = tuple(int(L[rank[k * (N_CORES * P)]]) for k in range(N_TILES))
    F = [P * lm for lm in lmax]
    Ftot = sum(F)

    xq = x.astype(ml_dtypes.float8_e4m3)              # [N_EX, D]

    in_maps = []
    for c in range(N_CORES):
        ex = sel[:, c]                                # [1024] example ids
        t_c = t[ex]
        node_c = paths[t_c]                           # [1024, MAX_LEN] int32
        code_c = codes[t_c]                           # [1024, MAX_LEN] f32
        L_c = L[ex]                                   # [1024]

        # --- x stationaries: [p, tile, c4, parity, e] ------------------
        xs = xq[ex].view(np.uint8).reshape(N_TILES, P, D)        # [k, e, d]
        xr = xs.reshape(N_TILES, P, 4, P, 2)                     # [k, e, c4, p, j]
        xst = np.ascontiguousarray(
            xr.transpose(3, 0, 2, 4, 1)                          # [p, k, c4, j, e]
        ).reshape(P, N_TILES * 1024).view(ml_dtypes.float8_e4m3)

        # --- pre-gathered transposed signed rows + bias ----------------
        Wt = np.empty((P, 8 * Ftot), dtype=np.uint8)
        bias = np.full((P, Ftot), OFF_BIAS, dtype=np.float32)
        f0 = 0
        for k in range(N_TILES):
            lm, Fk = lmax[k], F[k]
            rows = slice(k * P, (k + 1) * P)
            lv = np.minimum(L_c[rows], lm).astype(np.int64)      # [128]
            valid = np.arange(lm)[None, :] < lv[:, None]         # [128, lm]
            nodes = np.where(valid, node_c[rows, :lm], 0).reshape(Fk)
            signs = np.where(valid, -code_c[rows, :lm], 0.0).reshape(Fk)
            rowsW = (W[nodes] * (signs * W_SCALE)[:, None]).astype(
                ml_dtypes.float8_e4m3
            )                                                    # [Fk, D]
            # byte d = c4*256 + 2p + j  ->  [i, c4, p, j] -> [p, c4, i, j]
            rb = rowsW.view(np.uint8).reshape(Fk, 4, P, 2)
            Wt[:, 8 * f0 : 8 * (f0 + Fk)] = np.ascontiguousarray(
                rb.transpose(2, 1, 0, 3)                         # [p, c4, i, j]
            ).reshape(P, 8 * Fk)
            # bias rows: partition e' col (e,l): 0 iff e'==e and l < len(e')
            ev = np.repeat(np.arange(P), lm).reshape(P, lm)[valid]  # e of valid
            ci = np.arange(Fk).reshape(P, lm)[valid]                # col index
            bias[ev, f0 + ci] = 0.0
            f0 += Fk
        in_maps.append(
            {
                "Wt": Wt.view(ml_dtypes.float8_e4m3),
                "xst": xst,
                "bias": bias.astype(ml_dtypes.float8_e5m2),
            }
        )
    return lmax, in_maps


def kernel(x, W, t, paths, codes, lens):
    from concourse import bass_utils

    lmax, in_maps = _prepare(
        np.asarray(x), np.asarray(W), np.asarray(t),
        np.asarray(paths), np.asarray(codes), np.asarray(lens),
    )
    nc = _prog_cache.get(lmax)
    if nc is None:
        nc = _build_program(lmax)
        _prog_cache[lmax] = nc

    res = bass_utils.run_bass_kernel_spmd(nc, in_maps, core_ids=list(range(N_CORES)))
    total = sum(r["out"].astype(np.float64).sum() for r in res.results)
    return np.float32(total)


# revision 26
# speedup vs baseline: 1.1025x; 1.1025x over previous
"""Hierarchical-softmax loss kernel for Trainium2 (8 NeuronCores).

Strategy (v3: host-staged fp8 TensorE all-pairs)
------------------------------------------------
Data-parallel over n_ex: examples are globally sorted by path length and
dealt round-robin so every core sees 8 partition-tiles of 128 examples
with a near-common per-tile max length lm.

Per tile, the dot products are computed as a DENSE fp8 matmul on the
Tensor engine: stationary = the tile's 128 x-vectors (fp8, d on
partitions, parity-interleaved), moving = the tile's 128*lm signed path
rows (-code * 512 * W[node]).  The host pre-gathers those rows and
writes them to DRAM ALREADY in the transposed, parity-interleaved SBUF
byte layout the PE wants, so the device only runs plain full-rate
streaming DMAs (no indirect gather, no Q7 descriptor generation) and 4
DoubleRow fp8 matmuls per PSUM bank accumulate the full D=1024
contraction at 2 k-rows/cycle.

psum[e, col] holds x[e] . w_col for ALL (example, path-column) pairs of
the tile; only the block-diagonal (col belongs to e) is wanted. A bf16
bias table (0 on-path, -16384 off-path) added on the Vector engine sends
off-path entries to softplus(-32) ~ 1e-14, so the Scalar engine's
exp(v/512) -> ln(1+u) pass with free-axis accumulation per tile yields
the masked loss partials directly. Host sums the 128x8 partials/core.

All flops of the loss (dots, softplus, reduction) stay on device; the
host only reorders bytes. Per-core HBM traffic ~19MB, all streaming.
"""

import os
import sys

import numpy as np

for _p in ("/opt/trn_rl_repo", "/root/.axon_site/_ro/trn_rl_repo"):
    if os.path.isdir(_p) and _p not in sys.path:
        sys.path.append(_p)

V = 50257
N_DEC = V - 1
D = 1024
N_EX = 8192
MAX_LEN = 24
N_CORES = 8
P = 128
N_TILES = N_EX // (N_CORES * P)  # 8 example-tiles of 128 per core
W_SCALE = 512.0                  # fp8 pre-scale
OFF_BIAS = -16384.0              # off-path bias; /512 = -32 => softplus ~ 1e-14
GRP = 512                        # psum bank columns (fp32)
_prog_cache: dict = {}


def _patch_tail_drain(tile, mybir, bass_rust):
    """The pinned walrus encodes only a limited number of sync-waits per CTRL
    instruction, but Tile's kernel-tail Drain carries one wait per active
    processor lane. Spread the extra waits over single-wait NOPs."""
    if getattr(tile.TileContext._drain_and_barrier, "_split_waits", False):
        return

    def _drain_and_barrier(self, tick_clock, wait_clock):
        nc = self.nc
        drain_inst = nc.sync.drain()
        wait_clock.add_sem_waits(
            drain_inst.ins, bass_rust.ScopedClock({None: tick_clock.global_clock})
        )
        si = drain_inst.ins.sync_info
        waits = list(si.on_wait or [])
        if len(waits) > 1:
            si.on_wait = waits[:1]
            for w in waits[1:]:
                nop = nc.sync.nop(nofuse=True)
                nop.ins.sync_info = mybir.SyncInfo(on_wait=[w], on_update=[])
        nc.all_engine_barrier()
        popped = nc._tile_sem_poison_stack.pop()
        assert popped is self._sem_poison
        nc.clear_and_free_semaphores(list(self.sems.allocated().values()))
        nc.all_engine_barrier()

    _drain_and_barrier._split_waits = True
    tile.TileContext._drain_and_barrier = _drain_and_barrier


def _split_multiwait_instructions(nc, mybir, maxw=1):
    """Hoist extra sem-waits from any instruction onto single-wait NOPs placed
    immediately before it on the same engine (same aggregate wait semantics)."""
    f = nc.m.functions[0]
    tail = nc.cur_bb.bb
    blocks = list(f.blocks)
    if not any(b.name == tail.name for b in blocks):
        blocks.append(tail)
    for blk in blocks:
        snapshot = list(blk.instructions)
        heavy = [
            i for i in snapshot
            if i.sync_info and i.sync_info.on_wait and len(i.sync_info.on_wait) > maxw
        ]
        if not heavy:
            continue
        pre_len = len(tail.instructions)
        n_created = 0
        new_list = []
        for inst in snapshot:
            si = inst.sync_info
            if si and si.on_wait and len(si.on_wait) > maxw:
                waits = list(si.on_wait)
                extra, keep = waits[:-maxw], waits[-maxw:]
                si.on_wait = keep
                for w in extra:
                    nop = nc.engines[inst.engine].nop(nofuse=True)
                    nop.ins.sync_info = mybir.SyncInfo(on_wait=[w], on_update=[])
                    new_list.append(nop.ins)
                    n_created += 1
            new_list.append(inst)
        # builder appended the fresh NOPs to the tail block; strip them there
        t = list(tail.instructions)
        assert len(t) == pre_len + n_created
        if blk.name == tail.name:
            blk.instructions = new_list
        else:
            tail.instructions = t[:pre_len]
            blk.instructions = new_list


def _build_program(lmax: tuple):
    from concourse import bass, mybir
    import concourse.tile as tile
    import bass_rust

    _patch_tail_drain(tile, mybir, bass_rust)

    F = [P * lm for lm in lmax]        # pair-columns per tile
    Ftot = sum(F)
    # self-contained DMA/compute pieces of <=1024 columns
    pieces = []                        # (tile, col_start, col_count)
    for k in range(N_TILES):
        for j0 in range(0, F[k], 1024):
            pieces.append((k, j0, min(1024, F[k] - j0)))
    NPARTS = len(pieces)

    nc = bass.Bass("TRN2", target_bir_lowering=False)
    f32 = mybir.dt.float32
    bf16 = mybir.dt.bfloat16
    fp8 = mybir.dt.float8e4
    fp8b = mybir.dt.float8e5   # bias: {0, -16384}, both exact in e5m2

    # Wt: host-transposed signed rows; per partition p, tile k, the byte at
    # [c4*2*Fk + 2i + j] is W-element d = 2*(c4*128+p)+j of tile-k column i
    Wt = nc.declare_dram_parameter("Wt", [P, 8 * Ftot], fp8, isOutput=False)
    xst = nc.declare_dram_parameter("xst", [P, N_TILES * 1024], fp8, isOutput=False)
    bias = nc.declare_dram_parameter("bias", [P, Ftot], fp8b, isOutput=False)
    out = nc.declare_dram_parameter("out", [P, NPARTS], f32, isOutput=True)

    with tile.TileContext(nc) as tc:
        with (
            tc.tile_pool(name="meta", bufs=1) as meta,
            tc.tile_pool(name="gpool", bufs=1) as gpool,
            tc.tile_pool(name="bpool", bufs=1) as bpool,
            tc.tile_pool(name="vpool", bufs=2) as vpool,
            tc.tile_pool(name="spool", bufs=2) as spool,
            tc.psum_pool(name="pspool", bufs=1) as pspool,
            tc.tile_pool(name="outp", bufs=1) as outp,
        ):
            xst_t = meta.tile([P, N_TILES * 1024], fp8, tag="xst")
            nc.sync.dma_start(out=xst_t[:], in_=xst[:, :])

            parts = outp.tile([P, NPARTS], f32, tag="parts")
            f_off = [sum(F[:k]) for k in range(N_TILES)]

            # all DMAs on the (otherwise idle) sync HWDGE ring, issued
            # upfront in piece order; all tiles persistent in SBUF, so
            # the Wt stream runs gap-free and compute chases it at
            # piece granularity. Each tile's bias rides just before its
            # first Wt piece.
            gp, bt = {}, {}
            seen = set()
            for pi, (k, j0, pc) in enumerate(pieces):
                if k not in seen:
                    seen.add(k)
                    b = bpool.tile([P, F[k]], fp8b, tag=f"b{k}", name=f"b{k}")
                    nc.sync.dma_start(
                        out=b[:], in_=bias[:, f_off[k] : f_off[k] + F[k]]
                    )
                    bt[k] = b
                g = gpool.tile([P, 8 * pc], fp8, tag=f"g{pi}", name=f"g{pi}")
                w0 = 8 * (f_off[k] + j0)
                nc.sync.dma_start(out=g[:], in_=Wt[:, w0 : w0 + 8 * pc])
                gp[pi] = g

            for pi, (k, j0, pc) in enumerate(pieces):
                gflat = gp[pi]
                bias_t = bt[k]
                v = vpool.tile([P, pc], bf16, tag="v")
                for gi in range((pc + GRP - 1) // GRP):
                    h0 = gi * GRP
                    Fg = min(GRP, pc - h0)
                    ps = pspool.tile(
                        [P, GRP], f32, tag=f"ps{(2 * pi + gi) % 6}",
                        name=f"ps{(2 * pi + gi) % 6}",
                    )
                    for c in range(4):
                        rhs = gflat[
                            :, c * 2 * pc + 2 * h0 : c * 2 * pc + 2 * (h0 + Fg)
                        ].rearrange("p (f two) -> p two f", two=2)
                        lhsT = xst_t[
                            :, k * 1024 + c * 256 : k * 1024 + (c + 1) * 256
                        ].rearrange("p (two e) -> p two e", two=2)
                        nc.tensor.matmul(
                            out=ps[:, :Fg],
                            lhsT=lhsT,
                            rhs=rhs,
                            start=(c == 0),
                            stop=(c == 3),
                            perf_mode=mybir.MatmulPerfMode.DoubleRow,
                        )
                    nc.vector.tensor_tensor(
                        out=v[:, h0 : h0 + Fg],
                        in0=ps[:, :Fg],
                        in1=bias_t[:, j0 + h0 : j0 + h0 + Fg],
                        op=mybir.AluOpType.add,
                    )
                # softplus(v/512) = ln(1 + exp(v/512)); off-path v/512 = -32
                sp = spool.tile([P, 1024], bf16, tag="sp")
                nc.scalar.activation(
                    out=sp[:, :pc],
                    in_=v[:],
                    func=mybir.ActivationFunctionType.Exp,
                    scale=1.0 / W_SCALE,
                )
                sl = spool.tile([P, 1024], bf16, tag="sl")
                nc.scalar.activation(
                    out=sl[:, :pc],
                    in_=sp[:, :pc],
                    func=mybir.ActivationFunctionType.Ln,
                    bias=1.0,
                    accum_out=parts[:, pi : pi + 1],
                )
            nc.sync.dma_start(out=out[:, :], in_=parts[:])

    _split_multiwait_instructions(nc, mybir)
    return nc


def _prepare(x, W, t, paths, codes, lens):
    """Host-side prep: length-sorted round-robin shard; per-core pre-gathered,
    pre-signed, pre-transposed fp8 W path-rows; x stationaries; bias tables."""
    import ml_dtypes

    L = lens[t].astype(np.int64)                      # [N_EX]
    rank = np.argsort(-L, kind="stable")              # examples by length desc

    # slot s (0..1023) of core c takes example rank[s*8 + c]
    sel = rank.reshape(N_EX // N_CORES, N_CORES)      # [1024, 8]
    lmax = tuple(int(L[rank[k * (N_CORES * P)]]) for k in range(N_TILES))
    F = [P * lm for lm in lmax]
    Ftot = sum(F)

    xq = x.astype(ml_dtypes.float8_e4m3)              # [N_EX, D]

    in_maps = []
    for c in range(N_CORES):
        ex = sel[:, c]                                # [1024] example ids
        t_c = t[ex]
        node_c = paths[t_c]                           # [1024, MAX_LEN] int32
        code_c = codes[t_c]                           # [1024, MAX_LEN] f32
        L_c = L[ex]                                   # [1024]

        # --- x stationaries: [p, tile, c4, parity, e] ------------------
        xs = xq[ex].view(np.uint8).reshape(N_TILES, P, D)        # [k, e, d]
        xr = xs.reshape(N_TILES, P, 4, P, 2)                     # [k, e, c4, p, j]
        xst = np.ascontiguousarray(
            xr.transpose(3, 0, 2, 4, 1)                          # [p, k, c4, j, e]
        ).reshape(P, N_TILES * 1024).view(ml_dtypes.float8_e4m3)

        # --- pre-gathered transposed signed rows + bias ----------------
        Wt = np.empty((P, 8 * Ftot), dtype=np.uint8)
        bias = np.full((P, Ftot), OFF_BIAS, dtype=np.float32)
        f0 = 0
        for k in range(N_TILES):
            lm, Fk = lmax[k], F[k]
            rows = slice(k * P, (k + 1) * P)
            lv = np.minimum(L_c[rows], lm).astype(np.int64)      # [128]
            valid = np.arange(lm)[None, :] < lv[:, None]         # [128, lm]
            nodes = np.where(valid, node_c[rows, :lm], 0).reshape(Fk)
            signs = np.where(valid, -code_c[rows, :lm], 0.0).reshape(Fk)
            rowsW = (W[nodes] * (signs * W_SCALE)[:, None]).astype(
                ml_dtypes.float8_e4m3
            )                                                    # [Fk, D]
            # byte d = c4*256 + 2p + j -> per <=1024-col piece, contiguous
            # [p, c4, i, j] stripes (device streams pieces independently)
            for j0 in range(0, Fk, 1024):
                pc = min(1024, Fk - j0)
                rb = rowsW[j0 : j0 + pc].view(np.uint8).reshape(pc, 4, P, 2)
                Wt[:, 8 * (f0 + j0) : 8 * (f0 + j0 + pc)] = (
                    np.ascontiguousarray(rb.transpose(2, 1, 0, 3))
                    .reshape(P, 8 * pc)
                )
            # bias rows: partition e' col (e,l): 0 iff e'==e and l < len(e')
            ev = np.repeat(np.arange(P), lm).reshape(P, lm)[valid]  # e of valid
            ci = np.arange(Fk).reshape(P, lm)[valid]                # col index
            bias[ev, f0 + ci] = 0.0
            f0 += Fk
        in_maps.append(
            {
                "Wt": Wt.view(ml_dtypes.float8_e4m3),
                "xst": xst,
                "bias": bias.astype(ml_dtypes.float8_e5m2),
            }
        )
    return lmax, in_maps


def kernel(x, W, t, paths, codes, lens):
    from concourse import bass_utils

    lmax, in_maps = _prepare(
        np.asarray(x), np.asarray(W), np.asarray(t),
        np.asarray(paths), np.asarray(codes), np.asarray(lens),
    )
    nc = _prog_cache.get(lmax)
    if nc is None:
        nc = _build_program(lmax)
        _prog_cache[lmax] = nc

    res = bass_utils.run_bass_kernel_spmd(nc, in_maps, core_ids=list(range(N_CORES)))
    total = sum(r["out"].astype(np.float64).sum() for r in res.results)
    return np.float32(total)
